# revision 6
# baseline (speedup 1.0000x reference)
"""Bradley-Terry loss kernel for Trainium2 — Chebyshev/PE design, v3.

loss = sum_{i!=j} W[i,j] * softplus(b_j - b_i)
     = sum_{m,l} A[m,l] * z[m,l] - ln2 * trace(W),
  z[m,l] = sum_ij W_ij T_m(x_i) T_l(x_j),  x = (b - c)/h in [-1,1]

softplus(h*(y-x)) is approximated by a degree-63 tensor-product Chebyshev
expansion; the O(N^2) contraction is a matmul with the single-bf16 basis C
as stationary (M=64).  W is converted to bf16 on the HOST and uploaded at
half width — the kernel is HBM-read bound (~370 GB/s/core measured), so
halving W bytes halves the critical path.  End-to-end rel-err ~1.8e-5 vs
the fp64 reference (tolerance 2e-2), validated on CPU.

Per core, TensorE computes  Y[m, j] = sum_{i in shard} W[i, j] * T_m(x_i)
accumulating each 4096-column group across all 8 row-tiles in PSUM
([64, 512] x 8 banks).  W streams as 16 x 1MB contiguous-row bf16 DMAs on
the sync HWDGE queue feeding the matmuls directly (no on-chip cast);
ScalarE drains PSUM slab-by-slab with the y write for each slab issued
immediately so the tail pipelines.  Stage 2 (z = Y C, A-contraction) runs
in float64 on the host.
"""

import numpy as np
import ml_dtypes

import concourse.bacc as bacc
import concourse.bass as bass
import concourse.mybir as mybir
from concourse import tile
from concourse.bass_utils import run_bass_kernel_spmd

N = 8192
NCORES = 8
R = N // NCORES            # 1024 rows per core
P = 128                    # SBUF partitions
TROWS = R // P             # 8 row-tiles per core
CH = 4096                  # column group held live in PSUM per generation
NCH = N // CH              # 2 generations
SLAB = 512                 # PSUM bank free size (fp32)
NSLAB = CH // SLAB         # 8 banks per generation
DEG = 63
M1 = DEG + 1               # 64 chebyshev coefficients
_NEG_LN2 = -float(np.log(2.0))

_cached_nc = None


def _cheb_vals(x, deg):
    out = np.empty((len(x), deg + 1), dtype=np.float64)
    out[:, 0] = 1.0
    if deg >= 1:
        out[:, 1] = x
    for k in range(2, deg + 1):
        out[:, k] = 2 * x * out[:, k - 1] - out[:, k - 2]
    return out


def _cheb2d_coeffs(f, deg):
    n = deg + 1
    theta = (np.arange(n) + 0.5) * np.pi / n
    pts = np.cos(theta)
    F = f(pts[:, None], pts[None, :])
    Tm = np.cos(np.outer(np.arange(n), theta))
    A = (2.0 / n) * Tm @ F @ ((2.0 / n) * Tm).T
    A[0, :] /= 2
    A[:, 0] /= 2
    return A


def _build():
    nc = bacc.Bacc(
        "TRN2",
        target_bir_lowering=False,
        debug=False,
        enable_asserts=False,
        num_devices=NCORES,
    )
    f32 = mybir.dt.float32
    bf16 = mybir.dt.bfloat16
    w = nc.dram_tensor("w", [R, N], bf16, kind="ExternalInput")
    crows = nc.dram_tensor("crows", [P, TROWS * M1], bf16, kind="ExternalInput")
    diag = nc.dram_tensor("diag", [R], f32, kind="ExternalInput")
    y = nc.dram_tensor("y", [M1, N], f32, kind="ExternalOutput")
    dsum = nc.dram_tensor("dsum", [P, 1], f32, kind="ExternalOutput")

    with tile.TileContext(nc) as tc:
        with (
            tc.tile_pool(name="consts", bufs=1) as consts,
            tc.tile_pool(name="wpool", bufs=8) as wpool,
            tc.tile_pool(name="ypool", bufs=2) as ypool,
            tc.tile_pool(name="psum", bufs=1, space="PSUM") as pspool,
            tc.tile_pool(name="small", bufs=2) as small,
        ):
            crows_sb = consts.tile([P, TROWS * M1], bf16)
            nc.gpsimd.dma_start(crows_sb[:], crows.ap())
            diag_sb = consts.tile([P, TROWS], f32)
            nc.gpsimd.dma_start(diag_sb[:], diag.ap().rearrange("(t p) -> p t", p=P))

            # full-bank PSUM tiles so one tile == one bank (only rows
            # 0:M1 are written)
            ps = [
                pspool.tile([P, SLAB], f32, tag=f"ps{s}", name=f"ps{s}")
                for s in range(NSLAB)
            ]
            for ch in range(NCH):
                for t in range(TROWS):
                    wt = wpool.tile([P, CH], bf16, tag="w")
                    nc.sync.dma_start(
                        wt[:],
                        w.ap()[t * P : (t + 1) * P, ch * CH : (ch + 1) * CH],
                    )
                    lhsT = crows_sb[:, t * M1 : (t + 1) * M1]
                    # slabs 0-3 -> banks 0-3 on PE col-group 0 (psum rows
                    # 0:64); slabs 4-7 -> banks 4-7 on col-group 64 (psum
                    # rows 64:128).  Alternating halves makes consecutive
                    # matmuls land on disjoint PE column groups, so they
                    # execute concurrently instead of serializing on the
                    # fill+drain latency.
                    for b in range(NSLAB // 2):
                        nc.tensor.matmul(
                            ps[b][0:M1, :],
                            lhsT,
                            wt[:, b * SLAB : (b + 1) * SLAB],
                            start=(t == 0),
                            stop=(t == TROWS - 1),
                        )
                        s1 = NSLAB // 2 + b
                        nc.tensor.matmul(
                            ps[s1][M1 : 2 * M1, :],
                            lhsT,
                            wt[:, s1 * SLAB : (s1 + 1) * SLAB],
                            start=(t == 0),
                            stop=(t == TROWS - 1),
                        )
                yh = ypool.tile([M1, CH], f32, tag="y")
                for s in range(NSLAB):
                    # ScalarE drains PSUM slab-by-slab; each slab's y write
                    # goes out immediately so the tail pipelines
                    src = ps[s][0:M1, :] if s < NSLAB // 2 else ps[s][M1 : 2 * M1, :]
                    nc.scalar.copy(yh[:, s * SLAB : (s + 1) * SLAB], src)
                    dst = y.ap()[:, ch * CH + s * SLAB : ch * CH + (s + 1) * SLAB]
                    if ch == NCH - 1:
                        nc.sync.dma_start(dst, yh[:, s * SLAB : (s + 1) * SLAB])
                    else:
                        nc.gpsimd.dma_start(dst, yh[:, s * SLAB : (s + 1) * SLAB])

            # dsum[p] = -ln2 * sum_t diag[p, t]
            dscr = small.tile([P, TROWS], f32, tag="dscr")
            dacc = small.tile([P, 1], f32, tag="dacc")
            nc.vector.scalar_tensor_tensor(
                out=dscr[:],
                in0=diag_sb[:],
                scalar=_NEG_LN2,
                in1=diag_sb[:],
                op0=mybir.AluOpType.mult,
                op1=mybir.AluOpType.bypass,
                accum_out=dacc[:],
            )
            nc.gpsimd.dma_start(dsum.ap(), dacc[:])

    nc.compile()
    return nc


def _get_nc():
    global _cached_nc
    if _cached_nc is None:
        _cached_nc = _build()
    return _cached_nc


def kernel(win_matrix, betas, _trace=False):
    win_matrix = np.asarray(win_matrix, dtype=np.float32)
    betas = np.asarray(betas, dtype=np.float32)
    nc = _get_nc()

    b64 = betas.astype(np.float64)
    lo, hi = float(b64.min()), float(b64.max())
    c = 0.5 * (lo + hi)
    h = max(0.5 * (hi - lo) * 1.000001, 1e-12)
    x = (b64 - c) / h
    A = _cheb2d_coeffs(lambda X, Y: np.logaddexp(0.0, h * (Y - X)), DEG)
    C = _cheb_vals(x, DEG)                       # [N, 64] f64
    C_bf = C.astype(ml_dtypes.bfloat16)

    W_bf = win_matrix.astype(ml_dtypes.bfloat16)
    dvals = np.ascontiguousarray(np.diagonal(win_matrix))
    in_maps = []
    for cc in range(NCORES):
        rows = slice(cc * R, (cc + 1) * R)
        crows_np = np.ascontiguousarray(
            C_bf[rows].reshape(TROWS, P, M1).transpose(1, 0, 2).reshape(P, TROWS * M1)
        )
        in_maps.append(
            {
                "w": np.ascontiguousarray(W_bf[rows]),
                "crows": crows_np,
                "diag": np.ascontiguousarray(dvals[rows]),
            }
        )
    res = run_bass_kernel_spmd(
        nc, in_maps, core_ids=list(range(NCORES)), trace=_trace
    )

    Ysum = np.zeros((M1, N), dtype=np.float64)
    dtot = 0.0
    for cc in range(NCORES):
        Ysum += res.results[cc]["y"].astype(np.float64)
        dtot += float(res.results[cc]["dsum"].astype(np.float64).sum())
    z = Ysum @ C                                  # [64, 64]
    total = float((A * z).sum()) + dtot
    if _trace:
        kernel.last_results = res
    return np.array(total, dtype=np.float32)
